# revision 1
# baseline (speedup 1.0000x reference)
"""Trainium2 Bass kernel for nn_DecoderTreeNN (gather + segment_sum over trees).

Computes, for two embedding tables C_hop / C_hop1:
    out[t, seg, :] = sum_{i : tree_ids[i] == seg} C_t[token_ids[i], :]
returning [2, 32, 512, 128] f32.

Strategy (8 NeuronCores, SPMD):
  - 16384 segments -> 128 "windows" of 128 consecutive segments. Core c owns
    windows [16c, 16c+16); since tree_ids is sorted, each window's tokens are
    a contiguous slice of the token stream. Host pads every window to a fixed
    16384 token slots (max real occupancy of this distribution ~15.9k); pad
    tokens use id 0, whose embedding row is all-zero (padding_idx), so they
    contribute nothing.
  - Host concatenates the two tables into one [32000, 256] f32 table, so one
    gathered row (1024 B) serves both outputs.
  - On device, per 4096-token chunk one gpsimd.dma_gather pulls the rows into
    SBUF as [128, 32, 256] (token k = j*128 + p). Per 128-token tile j, the
    DVE builds a selection matrix S[p, s] = (tree_rel[p] == s) by comparing a
    per-partition scalar against an iota row; the PE accumulates
    S^T @ G  ->  PSUM[128 segs, 256] across the window's 128 tiles.
  - PSUM is copied to SBUF and DMA'd to a per-core [16, 128, 256] output;
    the host reassembles the full [2, 32, 512, 128].
"""

from contextlib import ExitStack

import ml_dtypes
import numpy as np

import concourse.bacc as bacc
import concourse.bass as bass
import concourse.mybir as mybir
import concourse.tile as tile
from concourse.bass_utils import run_bass_kernel_spmd
from concourse.library_config import mlp

P = 128
V = 32000
D = 128              # embedding dim per table
DD = 2 * D           # concatenated row width
N_CORES = 8
NSEG = 16384
SEGS_PW = 128        # segments per window
WG = NSEG // SEGS_PW             # 128 global windows
W = WG // N_CORES                # 16 windows per core
CAP = 16384                      # padded tokens per window
CHUNK = 2048                     # tokens per dma_gather
NCH = CAP // CHUNK               # 4 chunks per window
NJ = CHUNK // P                  # 32 token tiles per chunk
NQ = W * NCH                     # 64 chunks per core
SINGLE_PACKET = False            # dma_gather packetization mode (cap 1024 idxs)
GBUFS = 12                       # g-pool depth (gathers in flight)
NOREG = False                    # constant num_idxs (=CHUNK), no reg_load
SBUFS = 4                        # s-pool depth (DVE lookahead)

_compiled = None


def _build_program(reps=1, mode="full", n_queues=4):
    # mode: "full" | "gather_only" | "compute_only" | "gather_sbuf" — the
    # probe modes time sub-pipelines (outputs are garbage)
    do_gather = mode in ("full", "gather_only", "gather_sbuf", "full_nos")
    do_compute = mode in ("full", "compute_only", "full_nos")
    no_s = mode == "full_nos"
    sbuf_src = mode == "gather_sbuf"
    nc = bacc.Bacc(
        "TRN2",
        target_bir_lowering=False,
        debug=False,
        num_devices=N_CORES,
        num_swdge_queues=n_queues,
    )
    t_table = nc.dram_tensor("table", [V, DD], mybir.dt.bfloat16, kind="ExternalInput")
    if sbuf_src:
        # host-permuted copy: row v lives at [v % 128, (v // 128) * DD :]
        t_table2 = nc.dram_tensor(
            "table2", [P, (V // P) * DD], mybir.dt.bfloat16, kind="ExternalInput"
        )
    t_idx = nc.dram_tensor(
        "idx", [P, NQ * (CHUNK // 16)], mybir.dt.int16, kind="ExternalInput"
    )
    t_trel = nc.dram_tensor(
        "trel", [P, NQ * NJ], mybir.dt.float32, kind="ExternalInput"
    )
    t_cnt = nc.dram_tensor("cnt", [1, NQ], mybir.dt.int32, kind="ExternalInput")
    t_iota = nc.dram_tensor("iota", [P, P], mybir.dt.bfloat16, kind="ExternalInput")
    t_out = nc.dram_tensor(
        "out", [reps * W, P, DD], mybir.dt.float32, kind="ExternalOutput"
    )

    with tile.TileContext(nc) as tc, ExitStack() as ctx:
        const = ctx.enter_context(tc.tile_pool(name="const", bufs=1))
        # sbuf_src: the 16MB resident table squeezes the budget -> 2 bufs
        gbufs = 2 if sbuf_src else GBUFS
        gpool = ctx.enter_context(tc.tile_pool(name="g", bufs=gbufs))
        spool = ctx.enter_context(tc.tile_pool(name="s", bufs=SBUFS))
        opool = ctx.enter_context(tc.tile_pool(name="o", bufs=2))
        ppool = ctx.enter_context(tc.tile_pool(name="p", bufs=2, space="PSUM"))

        # Rotating per-chunk DMA sems. A single shared sem is unsound: SDMA
        # engines drain their rings independently, so a fast engine's incs
        # for later gathers could reach 16*(q+1) before a slow engine has
        # finished gather q. With a per-residue sem, each engine contributes
        # at most (q // N + 1) incs (per-engine ring order is FIFO and the
        # g-pool WAR edges keep issuance within bufs of consumption), so
        # value 16*(q // N + 1) proves every engine finished gather q.
        N_GSEMS = 8
        gsems = [nc.alloc_semaphore(f"gather_dma{i}") for i in range(N_GSEMS)]

        idx_all = const.tile([P, NQ * (CHUNK // 16)], mybir.dt.int16)
        nc.sync.dma_start(idx_all[:], t_idx[:])
        cnt_all = const.tile([1, NQ], mybir.dt.int32)
        nc.sync.dma_start(cnt_all[:], t_cnt[:])
        if not sbuf_src:
            trel_all = const.tile([P, NQ * NJ], mybir.dt.float32)
            nc.sync.dma_start(trel_all[:], t_trel[:])
            iota_t = const.tile([P, P], mybir.dt.bfloat16)
            nc.sync.dma_start(iota_t[:], t_iota[:])
        else:
            table_sb = const.tile([P, (V // P) * DD], mybir.dt.bfloat16)

        nc.gpsimd.load_library(mlp)

        if not do_gather:
            # compute_only probe: static pre-zeroed g buffers, no DMA
            gfix = [
                const.tile([P, NJ, DD], mybir.dt.bfloat16, name=f"gfix{i}")
                for i in range(3)
            ]
            for gt in gfix:
                nc.vector.memset(gt[:], 0.0)

        gctr = 0
        if do_gather and NOREG:
            # one shared count register: every chunk carries exactly CHUNK
            # descriptors (int-const num_idxs_reg is not a supported path)
            creg_const = nc.gpsimd.alloc_register("cnt_const")
            nc.gpsimd.reg_load(creg_const, cnt_all[0:1, 0:1])
        for r in range(reps):
            if sbuf_src:
                # per-rep resident-table refresh: one contiguous 16MB stream
                nc.sync.dma_start(table_sb[:], t_table2[:])
            for w in range(W):
                psum = ppool.tile([P, DD], mybir.dt.float32, space="PSUM")
                for c in range(NCH):
                    q = w * NCH + c
                    if not do_gather:
                        g = gfix[gctr % 3]
                    elif sbuf_src:
                        g = gpool.tile([P, DD // P, CHUNK], mybir.dt.bfloat16, tag="g")
                    else:
                        g = gpool.tile([P, NJ, DD], mybir.dt.bfloat16, tag="g")
                    if do_gather and gctr < gbufs and not sbuf_src and not NOREG:
                        # first rotation of each g slot: pad rows skipped by
                        # the negative-index trim would otherwise read
                        # uninitialized SBUF; NaN garbage poisons the matmul
                        # even under a zero selection row (0 * NaN = NaN)
                        nc.vector.memset(g[:], 0.0)
                    if do_gather:
                        # num_idxs_reg must carry the post-trim count: the
                        # ring reserves descriptors from the register value,
                        # and a mismatch with the trailing-negative trim
                        # corrupts the descriptor ring (device-fatal).
                        # NOREG: every chunk carries exactly CHUNK real
                        # descriptors, so the count is a compile-time const.
                        if NOREG:
                            creg = creg_const
                        else:
                            creg = nc.gpsimd.alloc_register(f"cnt{gctr}")
                            nc.gpsimd.reg_load(creg, cnt_all[0:1, q : q + 1])
                        idx_sl = idx_all[:, q * (CHUNK // 16) : (q + 1) * (CHUNK // 16)]
                        if sbuf_src:
                            nc.gpsimd.dma_gather(
                                g[:],
                                table_sb[:],
                                idx_sl,
                                CHUNK,
                                creg,
                                DD,
                                transpose=True,
                                sbuf_tokens_per_rank=P,
                                sbuf_free_dim_per_rank=DD * 2,  # bytes per row
                                sbuf_free_dim_pad_per_rank=0,
                                sbuf_byte_offset=0,
                                single_packet=SINGLE_PACKET,
                            ).then_inc(gsems[gctr % N_GSEMS], 16)
                        else:
                            nc.gpsimd.dma_gather(
                                g[:],
                                t_table[:],
                                idx_sl,
                                CHUNK,
                                creg,
                                DD,
                                # single-packet mode caps num_idxs at 16
                                # engines x 64 descs = 1024; beyond that the
                                # packet is malformed and wedges the device
                                single_packet=SINGLE_PACKET,
                                queue_num=gctr % n_queues,
                            ).then_inc(gsems[gctr % N_GSEMS], 16)
                    gctr += 1
                    if not do_compute:
                        continue
                    for j in range(NJ):
                        t = q * NJ + j
                        if no_s:
                            s = iota_t  # timing probe: constant lhsT
                        else:
                            s = spool.tile([P, P], mybir.dt.bfloat16, tag="s")
                            nc.vector.tensor_scalar(
                                out=s[:],
                                in0=iota_t[:],
                                scalar1=trel_all[:, t : t + 1],
                                scalar2=None,
                                op0=mybir.AluOpType.is_equal,
                            )
                        mm = nc.tensor.matmul(
                            out=psum[:],
                            lhsT=s[:],
                            rhs=g[:, j, :],
                            start=(c == 0 and j == 0),
                            stop=(c == NCH - 1 and j == NJ - 1),
                        )
                        if do_gather and j == 0:
                            mm._wait_ge(
                                gsems[(gctr - 1) % N_GSEMS],
                                16 * ((gctr - 1) // N_GSEMS + 1),
                            )
                if do_compute:
                    ot = opool.tile([P, DD], mybir.dt.float32, tag="o")
                    nc.vector.tensor_copy(out=ot[:], in_=psum[:])
                    nc.sync.dma_start(t_out[r * W + w], ot[:])
        if not do_compute:
            # drain: every gather's sem must reach its final value before the
            # program ends (no matmul consumers exist to wait on them)
            total = reps * W * NCH
            assert total % N_GSEMS == 0
            for i in range(N_GSEMS):
                nc.gpsimd.wait_ge(gsems[i], 16 * (total // N_GSEMS))

    nc.compile()
    return nc


def _pack_inputs(token_ids, tree_ids):
    tok = np.ascontiguousarray(np.asarray(token_ids, dtype=np.int32))
    tree = np.ascontiguousarray(np.asarray(tree_ids, dtype=np.int32))

    bounds = np.searchsorted(tree, np.arange(0, NSEG + 1, SEGS_PW))
    counts = np.diff(bounds)
    assert counts.max() <= CAP, f"window overflow: {counts.max()} > {CAP}"

    # pad slots: token -1 -> dma_gather skips the row entirely (trailing
    # negative indices are trimmed, saving the HBM traffic); tree_rel -1 ->
    # the selection row is all-zero so whatever is in the skipped SBUF row
    # contributes nothing
    tok_pad = np.full((WG, CAP), -1, dtype=np.int16)
    trel_pad = np.full((WG, CAP), -1.0, dtype=np.float32)
    for wg in range(WG):
        s, e = bounds[wg], bounds[wg + 1]
        n = e - s
        # sort the window's tokens by vocab id: the segment sum is
        # order-invariant (trel follows the permutation), and monotone gather
        # addresses turn the random 512B HBM reads into near-sequential ones
        order = np.argsort(tok[s:e], kind="stable")
        tok_pad[wg, :n] = tok[s:e][order].astype(np.int16)
        trel_pad[wg, :n] = (tree[s:e][order] - SEGS_PW * wg).astype(np.float32)

    # idx: per chunk, index k lives at [16g + k%16, k//16], replicated g=0..7
    idx = (
        tok_pad.reshape(N_CORES, W, NCH, CHUNK // 16, 16)
        .transpose(0, 4, 1, 2, 3)
        .reshape(N_CORES, 16, NQ * (CHUNK // 16))
    )
    idx = np.broadcast_to(idx[:, None, :, :], (N_CORES, 8, 16, NQ * (CHUNK // 16)))
    idx = np.ascontiguousarray(idx.reshape(N_CORES, P, NQ * (CHUNK // 16)))

    # trel: column t = q*NJ + j, row p -> token k = j*128 + p of chunk q
    trel = np.ascontiguousarray(
        trel_pad.reshape(N_CORES, W, NCH, NJ, P)
        .transpose(0, 4, 1, 2, 3)
        .reshape(N_CORES, P, NQ * NJ)
    )
    # real tokens per (core, chunk) for the runtime num_idxs register
    if NOREG:
        cnt = np.full((N_CORES, 1, NQ), CHUNK, dtype=np.int32)
    else:
        cnt = np.clip(
            counts.reshape(N_CORES, W, 1) - np.arange(NCH) * CHUNK, 0, CHUNK
        ).astype(np.int32)
        cnt = np.ascontiguousarray(cnt.reshape(N_CORES, 1, NQ))
    return idx, trel, cnt


def _prepare_in_maps(token_ids, tree_ids, C_hop, C_hop1):
    table = np.ascontiguousarray(
        np.concatenate(
            [np.asarray(C_hop, np.float32), np.asarray(C_hop1, np.float32)], axis=1
        ).astype(ml_dtypes.bfloat16)
    )
    idx, trel, cnt = _pack_inputs(token_ids, tree_ids)
    iota = np.ascontiguousarray(
        np.broadcast_to(
            np.arange(P, dtype=np.float32).astype(ml_dtypes.bfloat16), (P, P)
        )
    )
    return [
        {
            "table": table,
            "idx": idx[c],
            "trel": trel[c],
            "cnt": cnt[c],
            "iota": iota,
        }
        for c in range(N_CORES)
    ]


def kernel(token_ids, tree_ids, C_hop, C_hop1, batch_size, max_trees):
    global _compiled
    batch_size = int(batch_size)
    max_trees = int(max_trees)
    assert batch_size * max_trees == NSEG

    in_maps = _prepare_in_maps(token_ids, tree_ids, C_hop, C_hop1)

    if _compiled is None:
        _compiled = _build_program()
    nc = _compiled
    res = run_bass_kernel_spmd(nc, in_maps, core_ids=list(range(N_CORES)))

    # assemble: res[c]["out"][w, s, :] = concat row for segment 2048c + 128w + s
    allseg = np.concatenate(
        [res.results[c]["out"].reshape(W * P, DD) for c in range(N_CORES)], axis=0
    )  # [16384, 256]
    key = allseg[:, :D].reshape(batch_size, max_trees, D)
    val = allseg[:, D:].reshape(batch_size, max_trees, D)
    return np.stack([key, val]).astype(np.float32)



# revision 12
# speedup vs baseline: 2.5150x; 2.5150x over previous
"""Trainium2 Bass kernel for nn_DecoderTreeNN (gather + segment_sum over trees).

Computes, for two embedding tables C_hop / C_hop1:
    out[t, seg, :] = sum_{i : tree_ids[i] == seg} C_t[token_ids[i], :]
returning [2, 32, 512, 128] f32.

Strategy (8 NeuronCores, SPMD):
  - 16384 segments -> 128 "windows" of 128 consecutive segments. Core c owns
    windows [16c, 16c+16); since tree_ids is sorted, each window's tokens are
    a contiguous slice of the token stream. Host pads every window to a fixed
    16384 token slots; pad tokens use id 0, whose embedding row is all-zero
    (padding_idx), so they contribute nothing. Pads sit at the FRONT of the
    window so the sorted real tokens keep monotone gather addresses.
  - Host concatenates the two tables into one [32000, 256] bf16 table, so one
    gathered row (512 B) serves both outputs.
  - On device, per CHUNK-token chunk one gpsimd.dma_gather pulls the rows into
    SBUF as [128, NJ, 256] (token k = j*128 + p). Per chunk, ONE DVE
    tensor_tensor builds all NJ selection tiles S[p, j, s] =
    (tree_rel[p, j] == s) via broadcast APs (iota vs per-tile scalar); the PE
    accumulates S_j^T @ G_j -> PSUM[128 segs, 256] across the window's tiles.
  - PSUM is copied to SBUF (scalar engine) and DMA'd to a per-core
    [16, 128, 256] output; the host reassembles the full [2, 32, 512, 128].
"""

from contextlib import ExitStack

import ml_dtypes
import numpy as np

import concourse.bacc as bacc
import concourse.bass as bass
import concourse.mybir as mybir
import concourse.tile as tile
from concourse.bass_utils import run_bass_kernel_spmd
from concourse.library_config import mlp

P = 128
V = 32000
D = 128              # embedding dim per table
DD = 2 * D           # concatenated row width
N_CORES = 8
NSEG = 16384
SEGS_PW = 128        # segments per window
WG = NSEG // SEGS_PW             # 128 global windows
W = WG // N_CORES                # 16 windows per core
CAP = 16384                      # padded tokens per window
CHUNK = 2048                     # tokens per dma_gather
NCH = CAP // CHUNK               # chunks per window
NJ = CHUNK // P                  # token tiles per chunk
NQ = W * NCH                     # chunks per core
SINGLE_PACKET = False            # dma_gather packetization mode (cap 1024 idxs)
GBUFS = 9                        # g-pool depth (gathers in flight)
SBUFS = 3                        # s-pool depth (DVE lookahead, chunks)
N_GSEMS = 16                     # >= GBUFS so no two in-flight gathers share a sem

_compiled = None


def _build_program(reps=1, mode="full", n_queues=4, sbufs=SBUFS, gbufs_n=GBUFS,
                   nj=NJ, nch=NCH, chunk=CHUNK):
    # mode: "full" | "gather_only" | "compute_only" | "contend" — probe modes
    # time sub-pipelines (outputs are garbage). "contend" runs gather and
    # compute concurrently with no cross edges (compute reads static tiles).
    contend = mode == "contend"
    do_gather = mode in ("full", "gather_only", "contend")
    do_compute = mode in ("full", "compute_only", "contend")
    nq = W * nch
    nc = bacc.Bacc(
        "TRN2",
        target_bir_lowering=False,
        debug=False,
        num_devices=N_CORES,
        num_swdge_queues=n_queues,
    )
    t_table = nc.dram_tensor("table", [V, DD], mybir.dt.bfloat16, kind="ExternalInput")
    t_idx = nc.dram_tensor(
        "idx", [P, nq * (chunk // 16)], mybir.dt.int16, kind="ExternalInput"
    )
    t_trel = nc.dram_tensor(
        "trel", [P, nq * nj], mybir.dt.bfloat16, kind="ExternalInput"
    )
    t_cnt = nc.dram_tensor("cnt", [1, nq], mybir.dt.int32, kind="ExternalInput")
    t_iota = nc.dram_tensor("iota", [P, P], mybir.dt.bfloat16, kind="ExternalInput")
    t_out = nc.dram_tensor(
        "out", [reps * W, P, DD], mybir.dt.float32, kind="ExternalOutput"
    )

    with tile.TileContext(nc) as tc, ExitStack() as ctx:
        const = ctx.enter_context(tc.tile_pool(name="const", bufs=1))
        gpool = ctx.enter_context(tc.tile_pool(name="g", bufs=gbufs_n))
        spool = ctx.enter_context(tc.tile_pool(name="s", bufs=sbufs))
        opool = ctx.enter_context(tc.tile_pool(name="o", bufs=2))
        ppool = ctx.enter_context(tc.tile_pool(name="p", bufs=2, space="PSUM"))

        # One DMA sem per in-flight gather slot (rotating). N_GSEMS >= gbufs
        # guarantees the sem value 16*(q // N_GSEMS + 1) proves gather q is
        # fully drained on all 16 engines: the next user of the same sem
        # (gather q + N_GSEMS) cannot even be issued until gather q's
        # consumers ran (g-pool WAR), so no engine can contribute extra incs.
        assert N_GSEMS >= gbufs_n
        gsems = [nc.alloc_semaphore(f"gather_dma{i}") for i in range(N_GSEMS)]

        idx_all = const.tile([P, nq * (chunk // 16)], mybir.dt.int16)
        nc.sync.dma_start(idx_all[:], t_idx[:])
        cnt_all = const.tile([1, nq], mybir.dt.int32)
        nc.sync.dma_start(cnt_all[:], t_cnt[:])
        trel_all = const.tile([P, nq * nj], mybir.dt.bfloat16)
        nc.sync.dma_start(trel_all[:], t_trel[:])
        iota_t = const.tile([P, P], mybir.dt.bfloat16)
        nc.sync.dma_start(iota_t[:], t_iota[:])

        nc.gpsimd.load_library(mlp)

        if not do_gather or contend:
            # compute_only/contend probe: static pre-zeroed g buffers
            gfix = [
                const.tile([P, nj, DD], mybir.dt.bfloat16, name=f"gfix{i}")
                for i in range(3)
            ]
            for gt in gfix:
                nc.vector.memset(gt[:], 0.0)

        gctr = 0
        if do_gather:
            # every chunk carries exactly `chunk` real descriptors (pads are
            # token 0 whose row is all-zero), so one shared count register
            # suffices (int-const num_idxs_reg is not a supported path)
            creg_const = nc.gpsimd.alloc_register("cnt_const")
            nc.gpsimd.reg_load(creg_const, cnt_all[0:1, 0:1])
        for r in range(reps):
            for w in range(W):
                psum = ppool.tile([P, DD], mybir.dt.float32, space="PSUM")
                for c in range(nch):
                    q = w * nch + c
                    if not do_gather:
                        g = gfix[gctr % 3]
                    else:
                        g = gpool.tile([P, nj, DD], mybir.dt.bfloat16, tag="g")
                    if contend:
                        g_dma, g = g, gfix[gctr % 3]
                    else:
                        g_dma = g
                    if do_gather:
                        idx_sl = idx_all[:, q * (chunk // 16) : (q + 1) * (chunk // 16)]
                        nc.gpsimd.dma_gather(
                            g_dma[:],
                            t_table[:],
                            idx_sl,
                            chunk,
                            creg_const,
                            DD,
                            # single-packet mode caps num_idxs at 16
                            # engines x 64 descs = 1024; beyond that the
                            # packet is malformed and wedges the device
                            single_packet=SINGLE_PACKET,
                            queue_num=gctr % n_queues,
                        ).then_inc(gsems[gctr % N_GSEMS], 16)
                    gctr += 1
                    if not do_compute:
                        continue
                    # one DVE op builds all nj selection tiles of this chunk:
                    # s[p, j, t] = (iota[t] == trel[p, q*nj + j])
                    s = spool.tile([P, nj, P], mybir.dt.bfloat16, tag="s")
                    nc.vector.tensor_tensor(
                        out=s[:],
                        in0=iota_t[:].unsqueeze(1).broadcast_to((P, nj, P)),
                        in1=trel_all[:, q * nj : (q + 1) * nj]
                        .unsqueeze(2)
                        .broadcast_to((P, nj, P)),
                        op=mybir.AluOpType.is_equal,
                    )
                    for j in range(nj):
                        mm = nc.tensor.matmul(
                            out=psum[:],
                            lhsT=s[:, j, :],
                            rhs=g[:, j, :],
                            start=(c == 0 and j == 0),
                            stop=(c == nch - 1 and j == nj - 1),
                        )
                        if do_gather and not contend and j == 0:
                            mm._wait_ge(
                                gsems[(gctr - 1) % N_GSEMS],
                                16 * ((gctr - 1) // N_GSEMS + 1),
                            )
                if do_compute:
                    ot = opool.tile([P, DD], mybir.dt.float32, tag="o")
                    nc.scalar.copy(ot[:], psum[:])
                    nc.sync.dma_start(t_out[r * W + w], ot[:])
        if do_gather and (not do_compute or contend):
            # drain: every gather's sem must reach its final value before the
            # program ends (no matmul consumers exist to wait on them)
            total = reps * W * nch
            for i in range(N_GSEMS):
                n_i = total // N_GSEMS + (1 if i < total % N_GSEMS else 0)
                nc.gpsimd.wait_ge(gsems[i], 16 * n_i)

    nc.compile()
    return nc


def _pack_inputs(token_ids, tree_ids):
    tok = np.ascontiguousarray(np.asarray(token_ids, dtype=np.int32))
    tree = np.ascontiguousarray(np.asarray(tree_ids, dtype=np.int32))

    bounds = np.searchsorted(tree, np.arange(0, NSEG + 1, SEGS_PW))
    counts = np.diff(bounds)
    assert counts.max() <= CAP, f"window overflow: {counts.max()} > {CAP}"

    # pad slots: token 0 -> embedding row 0 is all-zero (padding_idx) so the
    # gathered row contributes nothing; tree_rel -1 -> selection row is
    # all-zero too. Pads go at the FRONT so the sorted real tokens keep
    # monotone (near-sequential) HBM gather addresses.
    tok_pad = np.zeros((WG, CAP), dtype=np.int16)
    trel_pad = np.full((WG, CAP), -1.0, dtype=np.float32)
    for wg in range(WG):
        s, e = bounds[wg], bounds[wg + 1]
        n = e - s
        # sort the window's tokens by vocab id: the segment sum is
        # order-invariant (trel follows the permutation), and monotone gather
        # addresses turn the random 512B HBM reads into near-sequential ones
        order = np.argsort(tok[s:e], kind="stable")
        tok_pad[wg, CAP - n :] = tok[s:e][order].astype(np.int16)
        trel_pad[wg, CAP - n :] = (tree[s:e][order] - SEGS_PW * wg).astype(np.float32)

    # idx: per chunk, index k lives at [16g + k%16, k//16], replicated g=0..7
    idx = (
        tok_pad.reshape(N_CORES, W, NCH, CHUNK // 16, 16)
        .transpose(0, 4, 1, 2, 3)
        .reshape(N_CORES, 16, NQ * (CHUNK // 16))
    )
    idx = np.broadcast_to(idx[:, None, :, :], (N_CORES, 8, 16, NQ * (CHUNK // 16)))
    idx = np.ascontiguousarray(idx.reshape(N_CORES, P, NQ * (CHUNK // 16)))

    # trel: column t = q*NJ + j, row p -> token k = j*128 + p of chunk q
    trel = np.ascontiguousarray(
        trel_pad.reshape(N_CORES, W, NCH, NJ, P)
        .transpose(0, 4, 1, 2, 3)
        .reshape(N_CORES, P, NQ * NJ)
        .astype(ml_dtypes.bfloat16)
    )
    # constant num_idxs: every chunk carries exactly CHUNK descriptors
    cnt = np.full((N_CORES, 1, NQ), CHUNK, dtype=np.int32)
    return idx, trel, cnt


def _prepare_in_maps(token_ids, tree_ids, C_hop, C_hop1):
    table = np.ascontiguousarray(
        np.concatenate(
            [np.asarray(C_hop, np.float32), np.asarray(C_hop1, np.float32)], axis=1
        ).astype(ml_dtypes.bfloat16)
    )
    idx, trel, cnt = _pack_inputs(token_ids, tree_ids)
    iota = np.ascontiguousarray(
        np.broadcast_to(
            np.arange(P, dtype=np.float32).astype(ml_dtypes.bfloat16), (P, P)
        )
    )
    return [
        {
            "table": table,
            "idx": idx[c],
            "trel": trel[c],
            "cnt": cnt[c],
            "iota": iota,
        }
        for c in range(N_CORES)
    ]


def kernel(token_ids, tree_ids, C_hop, C_hop1, batch_size, max_trees):
    global _compiled
    batch_size = int(batch_size)
    max_trees = int(max_trees)
    assert batch_size * max_trees == NSEG

    in_maps = _prepare_in_maps(token_ids, tree_ids, C_hop, C_hop1)

    if _compiled is None:
        _compiled = _build_program()
    nc = _compiled
    res = run_bass_kernel_spmd(nc, in_maps, core_ids=list(range(N_CORES)))

    # assemble: res[c]["out"][w, s, :] = concat row for segment 2048c + 128w + s
    allseg = np.concatenate(
        [res.results[c]["out"].reshape(W * P, DD) for c in range(N_CORES)], axis=0
    )  # [16384, 256]
    key = allseg[:, :D].reshape(batch_size, max_trees, D)
    val = allseg[:, D:].reshape(batch_size, max_trees, D)
    return np.stack([key, val]).astype(np.float32)
